# revision 1
# baseline (speedup 1.0000x reference)
"""Biquad IIR (DSVF) filter over x[512, 32768] on 8 trn2 NeuronCores.

Math: the filter y = lfilter(b, a, x) along time has poles strictly inside
the unit circle (g = tan(pi*sigmoid(.)/2) > 0, r = softplus(.) > 0), so the
impulse response h[k] decays geometrically.  Once |h[k]| falls below fp32
noise the IIR is exactly (to fp32) a K-tap FIR:

    y[t] = sum_{k<K} h[k] x[t-k]

With time on the SBUF partition axis in tiles of L=128, each output tile is
a sum of (D+1) Toeplitz matmuls against the current and D previous input
tiles, where D = ceil(K/L) - 1.  No recurrence survives on-device: every
tile is independent, so the 8 cores time-shard T with a D*L-sample halo.

Host side computes h (float64), the Toeplitz lhsT matrices, transposes x to
[T, B] so DMA loads [128 time, 512 batch] tiles are contiguous, and
transposes the result back.
"""

import math
import os

import numpy as np

B = 512
T = 32768
NCORES = 8
L = 128            # time-tile length == PE contract/partition dim
TPC = T // NCORES  # 4096 time steps per core
NT = TPC // L      # 32 output tiles per core

# matmul operand dtype: "float32" (exact, 4 cyc/row) or "float32r"
# (1 cyc/row at free-dim>=256, reduced-precision mode)
MM_DTYPE = os.environ.get("KERNEL_MM_DTYPE", "float32")
GROUP = 8          # PSUM banks used per matmul batch

# "fir16": truncated-FIR Toeplitz in fp16 hi/lo split arithmetic — 3 fp16
#          matmuls per weight-pass reproduce the fp32 product to ~2^-22
#          (fp16 products are exact in fp32 accumulation), at 1 PE cyc/row
#          instead of fp32's 4
# "chain": state-augmented single-matmul-per-tile IIR chain (exact fp32)
# "fir":   truncated-FIR Toeplitz in fp32, D+1 matmuls per tile
VARIANT = os.environ.get("KERNEL_VARIANT", "fir16")

LT = 126                      # chain variant: y-outputs per tile (+2 state rows)
CNT = -(-TPC // LT)           # 33 output tiles per core (last partial)
PADT = CNT * LT               # 4158 padded output rows per core
NCHAINS = 4

_cache: dict = {}


def _coeffs(g, r, m_hp, m_bp, m_lp):
    g = float(np.asarray(g).reshape(()))
    r = float(np.asarray(r).reshape(()))
    m_hp = float(np.asarray(m_hp).reshape(()))
    m_bp = float(np.asarray(m_bp).reshape(()))
    m_lp = float(np.asarray(m_lp).reshape(()))
    gg = math.tan(math.pi * (1.0 / (1.0 + math.exp(-g))) / 2.0)
    rr = math.log1p(math.exp(r))
    g2 = gg * gg
    b0 = g2 * m_lp + gg * m_bp + m_hp
    b1 = 2.0 * g2 * m_lp - 2.0 * m_hp
    b2 = g2 * m_lp - gg * m_bp + m_hp
    a0 = g2 + 2.0 * rr * gg + 1.0
    a1 = 2.0 * g2 - 2.0
    a2 = g2 - 2.0 * rr * gg + 1.0
    return b0 / a0, b1 / a0, b2 / a0, a1 / a0, a2 / a0


def _impulse_response(b0, b1, b2, a1, a2, n):
    """h[0..n-1] of the DF2T biquad, float64."""
    h = np.empty(n, np.float64)
    z1 = z2 = 0.0
    for t in range(n):
        xt = 1.0 if t == 0 else 0.0
        y = b0 * xt + z1
        z1, z2 = b1 * xt - a1 * y + z2, b2 * xt - a2 * y
        h[t] = y
    return h


def _plan(b0, b1, b2, a1, a2):
    """Returns (D, [lhsT_0 .. lhsT_D]) where lhsT_d[i, j] = h[j + d*L - i]
    (tap from input sample i of the d-tiles-back input tile to output j)."""
    hmax_n = 64 * L
    h = _impulse_response(b0, b1, b2, a1, a2, hmax_n)
    scale = np.max(np.abs(h))
    sig = np.nonzero(np.abs(h) > 1e-10 * scale)[0]
    last = int(sig[-1]) if len(sig) else 0
    if last >= hmax_n - 1:
        raise ValueError("impulse response decays too slowly for FIR plan")
    D = max(1, -(-(last + 1) // L) - 1)
    ws = []
    i = np.arange(L)[:, None]
    j = np.arange(L)[None, :]
    for d in range(D + 1):
        k = j + d * L - i
        w = np.where((k >= 0) & (k < hmax_n), h[np.clip(k, 0, hmax_n - 1)], 0.0)
        ws.append(np.ascontiguousarray(w, np.float32))
    return D, ws


def _build_fir16(D):
    import concourse.bacc as bacc
    import concourse.mybir as mybir
    import concourse.tile as tile
    from contextlib import ExitStack

    f32 = mybir.dt.float32
    f16 = mybir.dt.float16

    nc = bacc.Bacc("TRN2", target_bir_lowering=False, debug=False,
                   enable_asserts=False)
    # per row: cols 0:B = fp16 hi half of x, cols B:2B = fp16 lo half
    xhl = nc.dram_tensor("xhl", [TPC + D * L, 2 * B], f16,
                         kind="ExternalInput").ap()
    # all weight matrices side by side: wh0..whD, wl0..wlD
    wall = nc.dram_tensor("wall", [L, 2 * (D + 1) * L], f16,
                          kind="ExternalInput").ap()
    yt = nc.dram_tensor("yt", [TPC, B], f32, kind="ExternalOutput").ap()

    with ExitStack() as ctx:
        tc = ctx.enter_context(tile.TileContext(nc))
        wpool = ctx.enter_context(tc.tile_pool(name="wpool", bufs=1))
        # all x quads stay resident (8.7 MB) — loads never wait on slots
        xpool = ctx.enter_context(
            tc.tile_pool(name="xpool", bufs=(NT + D + 1) // 2))
        ypool = ctx.enter_context(tc.tile_pool(name="ypool", bufs=6))
        pspool = ctx.enter_context(
            tc.tile_pool(name="pspool", bufs=3, space="PSUM"))
        wrmpool = ctx.enter_context(
            tc.tile_pool(name="wrmpool", bufs=1, space="PSUM"))

        # all weights in one transfer on the scalar queue — it ramps in
        # parallel with the sync queue, so the x tiles lead sync and the
        # first matmul isn't serialized behind the weight transfer
        wt_all = wpool.tile([L, 2 * (D + 1) * L], f16, name="wt_all",
                            tag="wt_all")
        nc.scalar.dma_start(wt_all[:], wall[:])
        wht = [wt_all[:, d * L:(d + 1) * L] for d in range(D + 1)]
        wlt = [wt_all[:, (D + 1 + d) * L:(D + 2 + d) * L]
               for d in range(D + 1)]

        # first 4 tiles ride individual small DMAs so the matmul pipeline
        # can start as soon as tiles 0/1 land; the rest load as pairs
        groups = [[c] for c in range(4)]
        c = 4
        while c < NT + D:
            groups.append(list(range(c, min(c + 2, NT + D))))
            c += 2
        xhs, xls = [], []
        for gi, grp in enumerate(groups):
            n_sub = len(grp)
            t = xpool.tile([L, 2 * 2 * B], f16, name=f"xhl{gi}", tag="xhl")
            src = xhl[grp[0] * L:grp[0] * L + n_sub * L, :].rearrange(
                "(s p) e -> p s e", p=L)
            dst = t[:, 0:n_sub * 2 * B].rearrange("p (s e) -> p s e", s=n_sub)
            nc.sync.dma_start(dst, src)
            for s in range(n_sub):
                xhs.append(t[:, s * 2 * B:s * 2 * B + B])
                xls.append(t[:, s * 2 * B + B:(s + 1) * 2 * B])

        # ~3.5us of throwaway matmuls on a memset tile (no DMA dependency,
        # so they start during the queue ramp) keep the PE activity monitor
        # busy through the preroll — real matmuls then run at 2.4 GHz from
        # the start instead of warming up mid-kernel (HAM un-throttles
        # after ~3.4us of sustained activity)
        dummy = wpool.tile([L, 64], f16, name="dummy", tag="dummy")
        nc.gpsimd.memset(dummy[:], 0.0)
        wps = wrmpool.tile([64, 64], f32, name="wps", tag="wps")
        for _ in range(56):
            nc.tensor.matmul(wps[:], dummy[:], dummy[:],
                             start=True, stop=True)

        # tile-major: short 6-matmul accumulation spans keep PSUM banks and
        # x slots turning over.  PSUM tiles span 2 banks so one copy drains
        # two output tiles; 4 output tiles share one store DMA on the
        # GpSimd queue (so stores can't head-of-line block x loads on Sync)
        ytile = None
        for o2 in range(NT // 2):
            ps = pspool.tile([L, 2 * B], f32, name=f"ps{o2}", tag="ps")
            ytile = ypool.tile([L, 2 * B], f32, name=f"yt{o2}", tag="ytile")
            for half in range(2):
                o = o2 * 2 + half
                terms = []
                for d in range(D, -1, -1):
                    terms += [(wht[d], xhs[o + D - d]),
                              (wht[d], xls[o + D - d]),
                              (wlt[d], xhs[o + D - d])]
                for ti, (wt, xop) in enumerate(terms):
                    nc.tensor.matmul(
                        ps[:, half * B:(half + 1) * B], wt, xop,
                        start=(ti == 0), stop=(ti == len(terms) - 1),
                    )
            nc.vector.tensor_copy(ytile[:], ps[:])
            o0 = o2 * 2
            dst = yt[o0 * L:(o0 + 2) * L, :].rearrange("(s p) e -> p s e", p=L)
            src = ytile[:].rearrange("p (s e) -> p s e", s=2)
            # the final stores drain two queues wide (sync's loads are long
            # done by then)
            if o2 >= 13 and o2 % 2 == 1:
                nc.sync.dma_start(dst, src)
            else:
                nc.gpsimd.dma_start(dst, src)
    nc.compile()
    return nc


def _run_fir16(x, coeff_key, coeffs):
    from concourse.bass_utils import run_bass_kernel_spmd

    key = ("fir16",) + coeff_key
    if key not in _cache:
        D, ws = _plan(*coeffs)
        nc = _build_fir16(D)
        _cache[key] = (D, ws, nc)
    D, ws, nc = _cache[key]

    xt_pad = np.zeros((T + D * L, B), np.float32)
    xt_pad[D * L:] = x.T
    xhl = np.empty((T + D * L, 2 * B), np.float16)
    xhl[:, :B] = xt_pad
    xhl[:, B:] = xt_pad - xhl[:, :B].astype(np.float32)

    whs, wls = [], []
    for w in ws:
        wh = w.astype(np.float16)
        whs.append(wh)
        wls.append((w - wh.astype(np.float32)).astype(np.float16))
    wall = np.ascontiguousarray(np.concatenate(whs + wls, axis=1))

    in_maps = [
        {"xhl": np.ascontiguousarray(xhl[i * TPC:(i + 1) * TPC + D * L]),
         "wall": wall}
        for i in range(NCORES)
    ]

    res = run_bass_kernel_spmd(
        nc, in_maps, core_ids=list(range(NCORES)),
        trace=bool(int(os.environ.get("KERNEL_TRACE", "0"))),
        trace_cores=(list(range(NCORES))
                     if os.environ.get("KERNEL_TRACE_ALL") else None),
    )
    yt = np.concatenate([r["yt"] for r in res.results], axis=0)
    return res, np.ascontiguousarray(yt.T)


def _plan_chain(b0, b1, b2, a1, a2):
    """Augmented tile matrix W[128, 128] for the state-carrying chain.

    Contract index i: 0..125 = x within tile, 126/127 = incoming (z1, z2).
    Output index j:   0..125 = y within tile, 126/127 = outgoing (z1, z2).
    Built by basis simulation of the DF2T recurrence in float64."""
    nb = 128
    X = np.zeros((LT, nb))
    for k in range(LT):
        X[k, k] = 1.0
    z1 = np.zeros(nb)
    z2 = np.zeros(nb)
    z1[126] = 1.0
    z2[127] = 1.0
    Y = np.zeros((LT, nb))
    for t in range(LT):
        xt = X[t]
        y = b0 * xt + z1
        z1, z2 = b1 * xt - a1 * y + z2, b2 * xt - a2 * y
        Y[t] = y
    W = np.zeros((nb, nb))
    W[:, :LT] = Y.T
    W[:, 126] = z1
    W[:, 127] = z2
    # engines can't address a 2-partition AP at base 126, so permute the
    # state slots to partitions 0..1 (x/y occupy 2..127)
    p = np.r_[126, 127, 0:126]
    return np.ascontiguousarray(W[np.ix_(p, p)], np.float32)


def _build_chain(mm_dtype_name):
    import concourse.bacc as bacc
    import concourse.mybir as mybir
    import concourse.tile as tile
    from contextlib import ExitStack

    f32 = mybir.dt.float32
    mm_dt = getattr(mybir.dt, mm_dtype_name)

    nc = bacc.Bacc("TRN2", target_bir_lowering=False, debug=False,
                   enable_asserts=False)
    # rows 0..125: the 126 samples before this core's range (chain warm-up);
    # rows 126+t: own sample t (zero-padded past TPC up to PADT)
    xt = nc.dram_tensor("xt", [LT + PADT, B], mm_dt, kind="ExternalInput").ap()
    wd = nc.dram_tensor("w", [128, 128], mm_dt, kind="ExternalInput").ap()
    yt = nc.dram_tensor("yt", [PADT, B], f32, kind="ExternalOutput").ap()

    # chain k owns output tiles [bounds[k], bounds[k+1]); each chain starts
    # with one discarded warm-up tile at zero state
    base, rem = divmod(CNT, NCHAINS)
    bounds = [0]
    for k in range(NCHAINS):
        bounds.append(bounds[-1] + base + (1 if k < rem else 0))

    with ExitStack() as ctx:
        tc = ctx.enter_context(tile.TileContext(nc))
        wpool = ctx.enter_context(tc.tile_pool(name="wpool", bufs=1))
        xpool = ctx.enter_context(tc.tile_pool(name="xpool", bufs=10))
        ypool = ctx.enter_context(tc.tile_pool(name="ypool", bufs=6))
        pspool = ctx.enter_context(
            tc.tile_pool(name="pspool", bufs=8, space="PSUM"))

        wt = wpool.tile([128, 128], mm_dt, name="wt", tag="wt")
        nc.sync.dma_start(wt[:], wd[:])

        # steps[k] = list of (x_row_start, out_tile_or_None)
        steps = []
        for k in range(NCHAINS):
            s = [(bounds[k] * LT, None)]  # warm-up tile, y discarded
            for o in range(bounds[k], bounds[k + 1]):
                s.append((LT + o * LT, o))
            steps.append(s)

        prev_ps = [None] * NCHAINS
        for step in range(max(len(s) for s in steps)):
            for k in range(NCHAINS):
                if step >= len(steps[k]):
                    continue
                row0, o = steps[k][step]
                # rhs partitions: 0..1 = incoming state, 2..127 = x window
                rhs = xpool.tile([128, B], mm_dt, name=f"rhs{k}_{step}",
                                 tag="rhs")
                nc.sync.dma_start(rhs[2:128, :], xt[row0:row0 + LT, :])
                if step == 0:
                    nc.gpsimd.memset(rhs[0:2, :], 0.0)
                else:
                    nc.scalar.copy(rhs[0:2, :], prev_ps[k][0:2, :])
                ps = pspool.tile([128, B], f32, name=f"ps{k}_{step}", tag="ps")
                nc.tensor.matmul(ps[:], wt[:], rhs[:], start=True, stop=True)
                prev_ps[k] = ps
                if o is not None:
                    # psum partitions: 0..1 = outgoing state, 2..127 = y
                    ytile = ypool.tile([128, B], f32, name=f"yt{o}",
                                       tag="ytile")
                    nc.vector.tensor_copy(ytile[:], ps[:])
                    nc.sync.dma_start(yt[o * LT:(o + 1) * LT, :],
                                      ytile[2:128, :])
    nc.compile()
    return nc


def _build(D, mm_dtype_name):
    import concourse.bacc as bacc
    import concourse.mybir as mybir
    import concourse.tile as tile
    from contextlib import ExitStack

    f32 = mybir.dt.float32
    mm_dt = getattr(mybir.dt, mm_dtype_name)

    nc = bacc.Bacc("TRN2", target_bir_lowering=False, debug=False,
                   enable_asserts=False)
    xt = nc.dram_tensor("xt", [TPC + D * L, B], mm_dt, kind="ExternalInput").ap()
    wds = [
        nc.dram_tensor(f"w{d}", [L, L], mm_dt, kind="ExternalInput").ap()
        for d in range(D + 1)
    ]
    yt = nc.dram_tensor("yt", [TPC, B], f32, kind="ExternalOutput").ap()

    with ExitStack() as ctx:
        tc = ctx.enter_context(tile.TileContext(nc))
        wpool = ctx.enter_context(tc.tile_pool(name="wpool", bufs=1))
        xpool = ctx.enter_context(tc.tile_pool(name="xpool", bufs=14))
        ypool = ctx.enter_context(tc.tile_pool(name="ypool", bufs=6))
        pspool = ctx.enter_context(
            tc.tile_pool(name="pspool", bufs=GROUP, space="PSUM"))

        wts = []
        for d in range(D + 1):
            wtile = wpool.tile([L, L], mm_dt, name=f"wt{d}", tag=f"wt{d}")
            nc.sync.dma_start(wtile[:], wds[d][:])
            wts.append(wtile)

        xts = []
        for c in range(NT + D):
            xtile = xpool.tile([L, B], mm_dt, name=f"xt{c}", tag="xtile")
            nc.sync.dma_start(xtile[:], xt[c * L:(c + 1) * L, :])
            xts.append(xtile)

        for g0 in range(0, NT, GROUP):
            n = min(GROUP, NT - g0)
            pss = [
                pspool.tile([L, B], f32, name=f"ps{g0 + k}", tag="ps")
                for k in range(n)
            ]
            # weight-major: all matmuls sharing a stationary operand are
            # adjacent, accumulating across D+1 passes into n PSUM banks
            for d in range(D, -1, -1):
                for k in range(n):
                    o = g0 + k
                    nc.tensor.matmul(
                        pss[k][:],
                        wts[d][:],
                        xts[o + D - d][:],
                        start=(d == D),
                        stop=(d == 0),
                    )
            for k in range(n):
                o = g0 + k
                ytile = ypool.tile([L, B], f32, name=f"yt{o}", tag="ytile")
                # alternate drain between DVE and ACT so neither bottlenecks
                if k % 2 == 0:
                    nc.vector.tensor_copy(ytile[:], pss[k][:])
                else:
                    nc.scalar.copy(ytile[:], pss[k][:])
                nc.sync.dma_start(yt[o * L:(o + 1) * L, :], ytile[:])
    nc.compile()
    return nc


_last_results = None


def _run_fir(x, coeff_key, coeffs):
    from concourse.bass_utils import run_bass_kernel_spmd

    key = ("fir", MM_DTYPE) + coeff_key
    if key not in _cache:
        D, ws = _plan(*coeffs)
        nc = _build(D, MM_DTYPE)
        _cache[key] = (D, ws, nc)
    D, ws, nc = _cache[key]

    xt_pad = np.zeros((T + D * L, B), np.float32)
    xt_pad[D * L:] = x.T

    in_maps = []
    for i in range(NCORES):
        m = {"xt": np.ascontiguousarray(xt_pad[i * TPC:(i + 1) * TPC + D * L])}
        for d in range(D + 1):
            m[f"w{d}"] = ws[d]
        in_maps.append(m)

    res = run_bass_kernel_spmd(
        nc, in_maps, core_ids=list(range(NCORES)),
        trace=bool(int(os.environ.get("KERNEL_TRACE", "0"))),
        trace_cores=(list(range(NCORES))
                     if os.environ.get("KERNEL_TRACE_ALL") else None),
    )
    yt = np.concatenate([r["yt"] for r in res.results], axis=0)
    return res, np.ascontiguousarray(yt.T)


def _run_chain(x, coeff_key, coeffs):
    from concourse.bass_utils import run_bass_kernel_spmd

    key = ("chain", MM_DTYPE) + coeff_key
    if key not in _cache:
        w = _plan_chain(*coeffs)
        nc = _build_chain(MM_DTYPE)
        _cache[key] = (w, nc)
    w, nc = _cache[key]

    # global padded [LT warm-up + T + (PADT - TPC) tail, B]; core i reads
    # rows [i*TPC, i*TPC + LT + PADT) so each core sees its 126-sample
    # history and a zero/next-core tail (outputs there are discarded)
    xg = np.zeros((LT + T + (PADT - TPC), B), np.float32)
    xg[LT:LT + T] = x.T

    in_maps = [
        {"xt": np.ascontiguousarray(xg[i * TPC:i * TPC + LT + PADT]), "w": w}
        for i in range(NCORES)
    ]

    res = run_bass_kernel_spmd(
        nc, in_maps, core_ids=list(range(NCORES)),
        trace=bool(int(os.environ.get("KERNEL_TRACE", "0"))),
        trace_cores=(list(range(NCORES))
                     if os.environ.get("KERNEL_TRACE_ALL") else None),
    )
    yt = np.concatenate([r["yt"][:TPC] for r in res.results], axis=0)
    return res, np.ascontiguousarray(yt.T)


def kernel(x, g, r, m_hp, m_bp, m_lp):
    global _last_results
    b0, b1, b2, a1, a2 = _coeffs(g, r, m_hp, m_bp, m_lp)
    coeffs = (b0, b1, b2, a1, a2)
    coeff_key = tuple(round(c, 12) for c in coeffs)
    x = np.asarray(x, np.float32)
    if VARIANT == "fir16":
        res, y = _run_fir16(x, coeff_key, coeffs)
    elif VARIANT == "chain":
        res, y = _run_chain(x, coeff_key, coeffs)
    else:
        res, y = _run_fir(x, coeff_key, coeffs)
    _last_results = res
    return y



# revision 6
# speedup vs baseline: 1.7485x; 1.7485x over previous
"""Biquad IIR (DSVF) filter over x[512, 32768] on 8 trn2 NeuronCores.

Math: the filter y = lfilter(b, a, x) along time has poles strictly inside
the unit circle (g = tan(pi*sigmoid(.)/2) > 0, r = softplus(.) > 0), so the
impulse response h[k] decays geometrically.  Once |h[k]| falls below fp32
noise the IIR is exactly (to fp32) a K-tap FIR:

    y[t] = sum_{k<K} h[k] x[t-k]

With time on the SBUF partition axis in tiles of L=128, each output tile is
a sum of (D+1) Toeplitz matmuls against the current and D previous input
tiles, where D = ceil(K/L) - 1.  No recurrence survives on-device: every
tile is independent, so the 8 cores time-shard T with a D*L-sample halo.

Host side computes h (float64), the Toeplitz lhsT matrices, transposes x to
[T, B] so DMA loads [128 time, 512 batch] tiles are contiguous, and
transposes the result back.
"""

import math
import os

import numpy as np

B = 512
T = 32768
NCORES = 8
L = 128            # time-tile length == PE contract/partition dim
TPC = T // NCORES  # 4096 time steps per core
NT = TPC // L      # 32 output tiles per core

# matmul operand dtype: "float32" (exact, 4 cyc/row) or "float32r"
# (1 cyc/row at free-dim>=256, reduced-precision mode)
MM_DTYPE = os.environ.get("KERNEL_MM_DTYPE", "float32")
GROUP = 8          # PSUM banks used per matmul batch

# "fir16p": truncated-FIR Toeplitz entirely in fp16 (x, weights, and y all
#          fp16; fp32 PSUM accumulation).  Halves HBM traffic vs fir16's
#          hi/lo split at ~4e-4 relative error — far inside the 2e-2 gate
# "fir16": truncated-FIR Toeplitz in fp16 hi/lo split arithmetic — 3 fp16
#          matmuls per weight-pass reproduce the fp32 product to ~2^-22
#          (fp16 products are exact in fp32 accumulation), at 1 PE cyc/row
#          instead of fp32's 4
# "chain": state-augmented single-matmul-per-tile IIR chain (exact fp32)
# "fir":   truncated-FIR Toeplitz in fp32, D+1 matmuls per tile
VARIANT = os.environ.get("KERNEL_VARIANT", "fir16p")

LT = 126                      # chain variant: y-outputs per tile (+2 state rows)
CNT = -(-TPC // LT)           # 33 output tiles per core (last partial)
PADT = CNT * LT               # 4158 padded output rows per core
NCHAINS = 4

_cache: dict = {}


def _coeffs(g, r, m_hp, m_bp, m_lp):
    g = float(np.asarray(g).reshape(()))
    r = float(np.asarray(r).reshape(()))
    m_hp = float(np.asarray(m_hp).reshape(()))
    m_bp = float(np.asarray(m_bp).reshape(()))
    m_lp = float(np.asarray(m_lp).reshape(()))
    gg = math.tan(math.pi * (1.0 / (1.0 + math.exp(-g))) / 2.0)
    rr = math.log1p(math.exp(r))
    g2 = gg * gg
    b0 = g2 * m_lp + gg * m_bp + m_hp
    b1 = 2.0 * g2 * m_lp - 2.0 * m_hp
    b2 = g2 * m_lp - gg * m_bp + m_hp
    a0 = g2 + 2.0 * rr * gg + 1.0
    a1 = 2.0 * g2 - 2.0
    a2 = g2 - 2.0 * rr * gg + 1.0
    return b0 / a0, b1 / a0, b2 / a0, a1 / a0, a2 / a0


def _impulse_response(b0, b1, b2, a1, a2, n):
    """h[0..n-1] of the DF2T biquad, float64."""
    h = np.empty(n, np.float64)
    z1 = z2 = 0.0
    for t in range(n):
        xt = 1.0 if t == 0 else 0.0
        y = b0 * xt + z1
        z1, z2 = b1 * xt - a1 * y + z2, b2 * xt - a2 * y
        h[t] = y
    return h


def _plan(b0, b1, b2, a1, a2):
    """Returns (D, [lhsT_0 .. lhsT_D]) where lhsT_d[i, j] = h[j + d*L - i]
    (tap from input sample i of the d-tiles-back input tile to output j)."""
    hmax_n = 64 * L
    h = _impulse_response(b0, b1, b2, a1, a2, hmax_n)
    scale = np.max(np.abs(h))
    sig = np.nonzero(np.abs(h) > 1e-10 * scale)[0]
    last = int(sig[-1]) if len(sig) else 0
    if last >= hmax_n - 1:
        raise ValueError("impulse response decays too slowly for FIR plan")
    D = max(1, -(-(last + 1) // L) - 1)
    ws = []
    i = np.arange(L)[:, None]
    j = np.arange(L)[None, :]
    for d in range(D + 1):
        k = j + d * L - i
        w = np.where((k >= 0) & (k < hmax_n), h[np.clip(k, 0, hmax_n - 1)], 0.0)
        ws.append(np.ascontiguousarray(w, np.float32))
    return D, ws


def _mm_bases(ws):
    """Per-depth first-nonzero lhsT row, rounded down to a 32-partition
    boundary (engine APs address quadrant-aligned partition bases).  -1
    marks an all-zero weight matrix whose matmul is skipped."""
    bases = []
    for w in ws:
        nz = np.nonzero(np.any(w != 0.0, axis=1))[0]
        bases.append((int(nz[0]) // 32) * 32 if len(nz) else -1)
    return bases


def _build_fir16p(D, bases):
    import concourse.bacc as bacc
    import concourse.mybir as mybir
    import concourse.tile as tile
    from contextlib import ExitStack

    f32 = mybir.dt.float32
    f16 = mybir.dt.float16

    nc = bacc.Bacc("TRN2", target_bir_lowering=False, debug=False,
                   enable_asserts=False)
    xh = nc.dram_tensor("xh", [TPC + D * L, B], f16,
                        kind="ExternalInput").ap()
    wall = nc.dram_tensor("wall", [L, (D + 1) * L], f16,
                          kind="ExternalInput").ap()
    yt = nc.dram_tensor("yt", [TPC, B], f16, kind="ExternalOutput").ap()

    with ExitStack() as ctx:
        tc = ctx.enter_context(tile.TileContext(nc))
        wpool = ctx.enter_context(tc.tile_pool(name="wpool", bufs=1))
        # all x tiles stay resident (4.3 MB) — loads never wait on slots
        nload = 2 + -(-(NT + D - 2) // 4)
        xpool = ctx.enter_context(tc.tile_pool(name="xpool", bufs=nload))
        ypool = ctx.enter_context(tc.tile_pool(name="ypool", bufs=3))
        pspool = ctx.enter_context(
            tc.tile_pool(name="pspool", bufs=3, space="PSUM"))
        wrmpool = ctx.enter_context(
            tc.tile_pool(name="wrmpool", bufs=1, space="PSUM"))

        # weights ride the scalar queue so x tiles lead on sync
        wt_all = wpool.tile([L, (D + 1) * L], f16, name="wt_all",
                            tag="wt_all")
        nc.scalar.dma_start(wt_all[:], wall[:])
        wts = [wt_all[:, d * L:(d + 1) * L] for d in range(D + 1)]

        # first 2 tiles ride individual DMAs so the matmul pipeline starts
        # as soon as they land; the rest load as quads
        groups = [[0], [1]]
        c = 2
        while c < NT + D:
            groups.append(list(range(c, min(c + 4, NT + D))))
            c += 4
        xs = []
        for gi, grp in enumerate(groups):
            n_sub = len(grp)
            t = xpool.tile([L, 4 * B], f16, name=f"xg{gi}", tag="xg")
            src = xh[grp[0] * L:grp[0] * L + n_sub * L, :].rearrange(
                "(s p) e -> p s e", p=L)
            dst = t[:, 0:n_sub * B].rearrange("p (s e) -> p s e", s=n_sub)
            nc.sync.dma_start(dst, src)
            for s in range(n_sub):
                xs.append(t[:, s * B:(s + 1) * B])

        # ~3.5us of throwaway matmuls on a memset tile (no DMA dependency,
        # so they start during the queue ramp) keep the PE activity monitor
        # busy through the preroll — real matmuls then run at 2.4 GHz from
        # the start instead of warming up mid-kernel
        dummy = wpool.tile([L, 64], f16, name="dummy", tag="dummy")
        nc.gpsimd.memset(dummy[:], 0.0)
        wps = wrmpool.tile([64, 64], f32, name="wps", tag="wps")
        for _ in range(56):
            nc.tensor.matmul(wps[:], dummy[:], dummy[:],
                             start=True, stop=True)

        # 4 output tiles per SBUF ytile / store DMA; 2 per PSUM span.
        # Drains alternate DVE/ACT (Pool cannot read PSUM).
        drain = [nc.vector.tensor_copy, nc.scalar.copy]
        di = 0
        nq = NT // 4
        for q in range(nq):
            ytile = ypool.tile([L, 4 * B], f16, name=f"yt{q}", tag="ytile")
            for h2 in range(2):
                ps = pspool.tile([L, 2 * B], f32, name=f"ps{q}_{h2}",
                                 tag="ps")
                for half in range(2):
                    o = q * 4 + h2 * 2 + half
                    terms = [d for d in range(D, -1, -1) if bases[d] >= 0]
                    for ti, d in enumerate(terms):
                        b0 = bases[d]
                        nc.tensor.matmul(
                            ps[:, half * B:(half + 1) * B],
                            wts[d][b0:L, :],
                            xs[o + D - d][b0:L, :],
                            start=(ti == 0), stop=(ti == len(terms) - 1),
                        )
                drain[di % 2](ytile[:, h2 * 2 * B:(h2 + 1) * 2 * B], ps[:])
                di += 1
            o0 = q * 4
            dst = yt[o0 * L:(o0 + 4) * L, :].rearrange(
                "(s p) e -> p s e", p=L)
            src = ytile[:].rearrange("p (s e) -> p s e", s=4)
            # stores alternate Pool/ACT queues; the final ones drain on
            # sync, whose loads are long done by then
            if q >= nq - 2:
                nc.sync.dma_start(dst, src)
            elif q % 2 == 0:
                nc.gpsimd.dma_start(dst, src)
            else:
                nc.scalar.dma_start(dst, src)
    nc.compile()
    return nc


def _run_fir16p(x, coeff_key, coeffs):
    from concourse.bass_utils import run_bass_kernel_spmd

    key = ("fir16p",) + coeff_key
    if key not in _cache:
        D, ws = _plan(*coeffs)
        nc = _build_fir16p(D, _mm_bases(ws))
        _cache[key] = (D, ws, nc)
    D, ws, nc = _cache[key]

    xt_pad = np.zeros((T + D * L, B), np.float16)
    xt_pad[D * L:] = x.T
    wall = np.ascontiguousarray(
        np.concatenate([w.astype(np.float16) for w in ws], axis=1))

    in_maps = [
        {"xh": np.ascontiguousarray(xt_pad[i * TPC:(i + 1) * TPC + D * L]),
         "wall": wall}
        for i in range(NCORES)
    ]

    res = run_bass_kernel_spmd(
        nc, in_maps, core_ids=list(range(NCORES)),
        trace=bool(int(os.environ.get("KERNEL_TRACE", "0"))),
        trace_cores=(list(range(NCORES))
                     if os.environ.get("KERNEL_TRACE_ALL") else None),
    )
    yt = np.concatenate([r["yt"] for r in res.results], axis=0)
    return res, np.ascontiguousarray(yt.T.astype(np.float32))


def _build_fir16(D):
    import concourse.bacc as bacc
    import concourse.mybir as mybir
    import concourse.tile as tile
    from contextlib import ExitStack

    f32 = mybir.dt.float32
    f16 = mybir.dt.float16

    nc = bacc.Bacc("TRN2", target_bir_lowering=False, debug=False,
                   enable_asserts=False)
    # per row: cols 0:B = fp16 hi half of x, cols B:2B = fp16 lo half
    xhl = nc.dram_tensor("xhl", [TPC + D * L, 2 * B], f16,
                         kind="ExternalInput").ap()
    # all weight matrices side by side: wh0..whD, wl0..wlD
    wall = nc.dram_tensor("wall", [L, 2 * (D + 1) * L], f16,
                          kind="ExternalInput").ap()
    yt = nc.dram_tensor("yt", [TPC, B], f32, kind="ExternalOutput").ap()

    with ExitStack() as ctx:
        tc = ctx.enter_context(tile.TileContext(nc))
        wpool = ctx.enter_context(tc.tile_pool(name="wpool", bufs=1))
        # all x quads stay resident (8.7 MB) — loads never wait on slots
        xpool = ctx.enter_context(
            tc.tile_pool(name="xpool", bufs=(NT + D + 1) // 2))
        ypool = ctx.enter_context(tc.tile_pool(name="ypool", bufs=6))
        pspool = ctx.enter_context(
            tc.tile_pool(name="pspool", bufs=3, space="PSUM"))
        wrmpool = ctx.enter_context(
            tc.tile_pool(name="wrmpool", bufs=1, space="PSUM"))

        # all weights in one transfer on the scalar queue — it ramps in
        # parallel with the sync queue, so the x tiles lead sync and the
        # first matmul isn't serialized behind the weight transfer
        wt_all = wpool.tile([L, 2 * (D + 1) * L], f16, name="wt_all",
                            tag="wt_all")
        nc.scalar.dma_start(wt_all[:], wall[:])
        wht = [wt_all[:, d * L:(d + 1) * L] for d in range(D + 1)]
        wlt = [wt_all[:, (D + 1 + d) * L:(D + 2 + d) * L]
               for d in range(D + 1)]

        # first 4 tiles ride individual small DMAs so the matmul pipeline
        # can start as soon as tiles 0/1 land; the rest load as pairs
        groups = [[c] for c in range(4)]
        c = 4
        while c < NT + D:
            groups.append(list(range(c, min(c + 2, NT + D))))
            c += 2
        xhs, xls = [], []
        for gi, grp in enumerate(groups):
            n_sub = len(grp)
            t = xpool.tile([L, 2 * 2 * B], f16, name=f"xhl{gi}", tag="xhl")
            src = xhl[grp[0] * L:grp[0] * L + n_sub * L, :].rearrange(
                "(s p) e -> p s e", p=L)
            dst = t[:, 0:n_sub * 2 * B].rearrange("p (s e) -> p s e", s=n_sub)
            nc.sync.dma_start(dst, src)
            for s in range(n_sub):
                xhs.append(t[:, s * 2 * B:s * 2 * B + B])
                xls.append(t[:, s * 2 * B + B:(s + 1) * 2 * B])

        # ~3.5us of throwaway matmuls on a memset tile (no DMA dependency,
        # so they start during the queue ramp) keep the PE activity monitor
        # busy through the preroll — real matmuls then run at 2.4 GHz from
        # the start instead of warming up mid-kernel (HAM un-throttles
        # after ~3.4us of sustained activity)
        dummy = wpool.tile([L, 64], f16, name="dummy", tag="dummy")
        nc.gpsimd.memset(dummy[:], 0.0)
        wps = wrmpool.tile([64, 64], f32, name="wps", tag="wps")
        for _ in range(56):
            nc.tensor.matmul(wps[:], dummy[:], dummy[:],
                             start=True, stop=True)

        # tile-major: short 6-matmul accumulation spans keep PSUM banks and
        # x slots turning over.  PSUM tiles span 2 banks so one copy drains
        # two output tiles; 4 output tiles share one store DMA on the
        # GpSimd queue (so stores can't head-of-line block x loads on Sync)
        ytile = None
        for o2 in range(NT // 2):
            ps = pspool.tile([L, 2 * B], f32, name=f"ps{o2}", tag="ps")
            ytile = ypool.tile([L, 2 * B], f32, name=f"yt{o2}", tag="ytile")
            for half in range(2):
                o = o2 * 2 + half
                terms = []
                for d in range(D, -1, -1):
                    terms += [(wht[d], xhs[o + D - d]),
                              (wht[d], xls[o + D - d]),
                              (wlt[d], xhs[o + D - d])]
                for ti, (wt, xop) in enumerate(terms):
                    nc.tensor.matmul(
                        ps[:, half * B:(half + 1) * B], wt, xop,
                        start=(ti == 0), stop=(ti == len(terms) - 1),
                    )
            nc.vector.tensor_copy(ytile[:], ps[:])
            o0 = o2 * 2
            dst = yt[o0 * L:(o0 + 2) * L, :].rearrange("(s p) e -> p s e", p=L)
            src = ytile[:].rearrange("p (s e) -> p s e", s=2)
            # the final stores drain two queues wide (sync's loads are long
            # done by then)
            if o2 >= 13 and o2 % 2 == 1:
                nc.sync.dma_start(dst, src)
            else:
                nc.gpsimd.dma_start(dst, src)
    nc.compile()
    return nc


def _run_fir16(x, coeff_key, coeffs):
    from concourse.bass_utils import run_bass_kernel_spmd

    key = ("fir16",) + coeff_key
    if key not in _cache:
        D, ws = _plan(*coeffs)
        nc = _build_fir16(D)
        _cache[key] = (D, ws, nc)
    D, ws, nc = _cache[key]

    xt_pad = np.zeros((T + D * L, B), np.float32)
    xt_pad[D * L:] = x.T
    xhl = np.empty((T + D * L, 2 * B), np.float16)
    xhl[:, :B] = xt_pad
    xhl[:, B:] = xt_pad - xhl[:, :B].astype(np.float32)

    whs, wls = [], []
    for w in ws:
        wh = w.astype(np.float16)
        whs.append(wh)
        wls.append((w - wh.astype(np.float32)).astype(np.float16))
    wall = np.ascontiguousarray(np.concatenate(whs + wls, axis=1))

    in_maps = [
        {"xhl": np.ascontiguousarray(xhl[i * TPC:(i + 1) * TPC + D * L]),
         "wall": wall}
        for i in range(NCORES)
    ]

    res = run_bass_kernel_spmd(
        nc, in_maps, core_ids=list(range(NCORES)),
        trace=bool(int(os.environ.get("KERNEL_TRACE", "0"))),
        trace_cores=(list(range(NCORES))
                     if os.environ.get("KERNEL_TRACE_ALL") else None),
    )
    yt = np.concatenate([r["yt"] for r in res.results], axis=0)
    return res, np.ascontiguousarray(yt.T)


def _plan_chain(b0, b1, b2, a1, a2):
    """Augmented tile matrix W[128, 128] for the state-carrying chain.

    Contract index i: 0..125 = x within tile, 126/127 = incoming (z1, z2).
    Output index j:   0..125 = y within tile, 126/127 = outgoing (z1, z2).
    Built by basis simulation of the DF2T recurrence in float64."""
    nb = 128
    X = np.zeros((LT, nb))
    for k in range(LT):
        X[k, k] = 1.0
    z1 = np.zeros(nb)
    z2 = np.zeros(nb)
    z1[126] = 1.0
    z2[127] = 1.0
    Y = np.zeros((LT, nb))
    for t in range(LT):
        xt = X[t]
        y = b0 * xt + z1
        z1, z2 = b1 * xt - a1 * y + z2, b2 * xt - a2 * y
        Y[t] = y
    W = np.zeros((nb, nb))
    W[:, :LT] = Y.T
    W[:, 126] = z1
    W[:, 127] = z2
    # engines can't address a 2-partition AP at base 126, so permute the
    # state slots to partitions 0..1 (x/y occupy 2..127)
    p = np.r_[126, 127, 0:126]
    return np.ascontiguousarray(W[np.ix_(p, p)], np.float32)


def _build_chain(mm_dtype_name):
    import concourse.bacc as bacc
    import concourse.mybir as mybir
    import concourse.tile as tile
    from contextlib import ExitStack

    f32 = mybir.dt.float32
    mm_dt = getattr(mybir.dt, mm_dtype_name)

    nc = bacc.Bacc("TRN2", target_bir_lowering=False, debug=False,
                   enable_asserts=False)
    # rows 0..125: the 126 samples before this core's range (chain warm-up);
    # rows 126+t: own sample t (zero-padded past TPC up to PADT)
    xt = nc.dram_tensor("xt", [LT + PADT, B], mm_dt, kind="ExternalInput").ap()
    wd = nc.dram_tensor("w", [128, 128], mm_dt, kind="ExternalInput").ap()
    yt = nc.dram_tensor("yt", [PADT, B], f32, kind="ExternalOutput").ap()

    # chain k owns output tiles [bounds[k], bounds[k+1]); each chain starts
    # with one discarded warm-up tile at zero state
    base, rem = divmod(CNT, NCHAINS)
    bounds = [0]
    for k in range(NCHAINS):
        bounds.append(bounds[-1] + base + (1 if k < rem else 0))

    with ExitStack() as ctx:
        tc = ctx.enter_context(tile.TileContext(nc))
        wpool = ctx.enter_context(tc.tile_pool(name="wpool", bufs=1))
        xpool = ctx.enter_context(tc.tile_pool(name="xpool", bufs=10))
        ypool = ctx.enter_context(tc.tile_pool(name="ypool", bufs=6))
        pspool = ctx.enter_context(
            tc.tile_pool(name="pspool", bufs=8, space="PSUM"))

        wt = wpool.tile([128, 128], mm_dt, name="wt", tag="wt")
        nc.sync.dma_start(wt[:], wd[:])

        # steps[k] = list of (x_row_start, out_tile_or_None)
        steps = []
        for k in range(NCHAINS):
            s = [(bounds[k] * LT, None)]  # warm-up tile, y discarded
            for o in range(bounds[k], bounds[k + 1]):
                s.append((LT + o * LT, o))
            steps.append(s)

        prev_ps = [None] * NCHAINS
        for step in range(max(len(s) for s in steps)):
            for k in range(NCHAINS):
                if step >= len(steps[k]):
                    continue
                row0, o = steps[k][step]
                # rhs partitions: 0..1 = incoming state, 2..127 = x window
                rhs = xpool.tile([128, B], mm_dt, name=f"rhs{k}_{step}",
                                 tag="rhs")
                nc.sync.dma_start(rhs[2:128, :], xt[row0:row0 + LT, :])
                if step == 0:
                    nc.gpsimd.memset(rhs[0:2, :], 0.0)
                else:
                    nc.scalar.copy(rhs[0:2, :], prev_ps[k][0:2, :])
                ps = pspool.tile([128, B], f32, name=f"ps{k}_{step}", tag="ps")
                nc.tensor.matmul(ps[:], wt[:], rhs[:], start=True, stop=True)
                prev_ps[k] = ps
                if o is not None:
                    # psum partitions: 0..1 = outgoing state, 2..127 = y
                    ytile = ypool.tile([128, B], f32, name=f"yt{o}",
                                       tag="ytile")
                    nc.vector.tensor_copy(ytile[:], ps[:])
                    nc.sync.dma_start(yt[o * LT:(o + 1) * LT, :],
                                      ytile[2:128, :])
    nc.compile()
    return nc


def _build(D, mm_dtype_name):
    import concourse.bacc as bacc
    import concourse.mybir as mybir
    import concourse.tile as tile
    from contextlib import ExitStack

    f32 = mybir.dt.float32
    mm_dt = getattr(mybir.dt, mm_dtype_name)

    nc = bacc.Bacc("TRN2", target_bir_lowering=False, debug=False,
                   enable_asserts=False)
    xt = nc.dram_tensor("xt", [TPC + D * L, B], mm_dt, kind="ExternalInput").ap()
    wds = [
        nc.dram_tensor(f"w{d}", [L, L], mm_dt, kind="ExternalInput").ap()
        for d in range(D + 1)
    ]
    yt = nc.dram_tensor("yt", [TPC, B], f32, kind="ExternalOutput").ap()

    with ExitStack() as ctx:
        tc = ctx.enter_context(tile.TileContext(nc))
        wpool = ctx.enter_context(tc.tile_pool(name="wpool", bufs=1))
        xpool = ctx.enter_context(tc.tile_pool(name="xpool", bufs=14))
        ypool = ctx.enter_context(tc.tile_pool(name="ypool", bufs=6))
        pspool = ctx.enter_context(
            tc.tile_pool(name="pspool", bufs=GROUP, space="PSUM"))

        wts = []
        for d in range(D + 1):
            wtile = wpool.tile([L, L], mm_dt, name=f"wt{d}", tag=f"wt{d}")
            nc.sync.dma_start(wtile[:], wds[d][:])
            wts.append(wtile)

        xts = []
        for c in range(NT + D):
            xtile = xpool.tile([L, B], mm_dt, name=f"xt{c}", tag="xtile")
            nc.sync.dma_start(xtile[:], xt[c * L:(c + 1) * L, :])
            xts.append(xtile)

        for g0 in range(0, NT, GROUP):
            n = min(GROUP, NT - g0)
            pss = [
                pspool.tile([L, B], f32, name=f"ps{g0 + k}", tag="ps")
                for k in range(n)
            ]
            # weight-major: all matmuls sharing a stationary operand are
            # adjacent, accumulating across D+1 passes into n PSUM banks
            for d in range(D, -1, -1):
                for k in range(n):
                    o = g0 + k
                    nc.tensor.matmul(
                        pss[k][:],
                        wts[d][:],
                        xts[o + D - d][:],
                        start=(d == D),
                        stop=(d == 0),
                    )
            for k in range(n):
                o = g0 + k
                ytile = ypool.tile([L, B], f32, name=f"yt{o}", tag="ytile")
                # alternate drain between DVE and ACT so neither bottlenecks
                if k % 2 == 0:
                    nc.vector.tensor_copy(ytile[:], pss[k][:])
                else:
                    nc.scalar.copy(ytile[:], pss[k][:])
                nc.sync.dma_start(yt[o * L:(o + 1) * L, :], ytile[:])
    nc.compile()
    return nc


_last_results = None


def _run_fir(x, coeff_key, coeffs):
    from concourse.bass_utils import run_bass_kernel_spmd

    key = ("fir", MM_DTYPE) + coeff_key
    if key not in _cache:
        D, ws = _plan(*coeffs)
        nc = _build(D, MM_DTYPE)
        _cache[key] = (D, ws, nc)
    D, ws, nc = _cache[key]

    xt_pad = np.zeros((T + D * L, B), np.float32)
    xt_pad[D * L:] = x.T

    in_maps = []
    for i in range(NCORES):
        m = {"xt": np.ascontiguousarray(xt_pad[i * TPC:(i + 1) * TPC + D * L])}
        for d in range(D + 1):
            m[f"w{d}"] = ws[d]
        in_maps.append(m)

    res = run_bass_kernel_spmd(
        nc, in_maps, core_ids=list(range(NCORES)),
        trace=bool(int(os.environ.get("KERNEL_TRACE", "0"))),
        trace_cores=(list(range(NCORES))
                     if os.environ.get("KERNEL_TRACE_ALL") else None),
    )
    yt = np.concatenate([r["yt"] for r in res.results], axis=0)
    return res, np.ascontiguousarray(yt.T)


def _run_chain(x, coeff_key, coeffs):
    from concourse.bass_utils import run_bass_kernel_spmd

    key = ("chain", MM_DTYPE) + coeff_key
    if key not in _cache:
        w = _plan_chain(*coeffs)
        nc = _build_chain(MM_DTYPE)
        _cache[key] = (w, nc)
    w, nc = _cache[key]

    # global padded [LT warm-up + T + (PADT - TPC) tail, B]; core i reads
    # rows [i*TPC, i*TPC + LT + PADT) so each core sees its 126-sample
    # history and a zero/next-core tail (outputs there are discarded)
    xg = np.zeros((LT + T + (PADT - TPC), B), np.float32)
    xg[LT:LT + T] = x.T

    in_maps = [
        {"xt": np.ascontiguousarray(xg[i * TPC:i * TPC + LT + PADT]), "w": w}
        for i in range(NCORES)
    ]

    res = run_bass_kernel_spmd(
        nc, in_maps, core_ids=list(range(NCORES)),
        trace=bool(int(os.environ.get("KERNEL_TRACE", "0"))),
        trace_cores=(list(range(NCORES))
                     if os.environ.get("KERNEL_TRACE_ALL") else None),
    )
    yt = np.concatenate([r["yt"][:TPC] for r in res.results], axis=0)
    return res, np.ascontiguousarray(yt.T)


def kernel(x, g, r, m_hp, m_bp, m_lp):
    global _last_results
    b0, b1, b2, a1, a2 = _coeffs(g, r, m_hp, m_bp, m_lp)
    coeffs = (b0, b1, b2, a1, a2)
    coeff_key = tuple(round(c, 12) for c in coeffs)
    x = np.asarray(x, np.float32)
    if VARIANT == "fir16p":
        res, y = _run_fir16p(x, coeff_key, coeffs)
    elif VARIANT == "fir16":
        res, y = _run_fir16(x, coeff_key, coeffs)
    elif VARIANT == "chain":
        res, y = _run_chain(x, coeff_key, coeffs)
    else:
        res, y = _run_fir(x, coeff_key, coeffs)
    _last_results = res
    return y

